# revision 2
# baseline (speedup 1.0000x reference)
"""Trainium2 Bass kernel for a continuous-convolution layer (v2).

Math: out[b,x,c] = (1/S) * sum_s f(||x_bx - y_bs||^2) * u[b,s,c]
where f is a fixed scalar->scalar MLP (width 16, depth 4, tanh residual
blocks) whose weights arrive as runtime inputs.

Strategy (v2):
  * f(r) ~ c0 + c1*r + a0*q + sum_k a_k*relu(q - theta_k),  q = exp(-s0*r).
    The affine part is computed exactly on the host (rank-2 over sensors).
    The nonlinear part needs ONE ACT exp per batch (the PSUM->SBUF bf16
    gateway) plus (W-1) DVE tensor_scalar relu ops per batch, which run in
    the 4x DVE perf mode (bf16, SBUF).  Knots/scale/amplitudes are fit at
    runtime against the exact f with the bf16 pipeline simulated on host.
  * r is produced per batch by a K=4 bf16 matmul r = ly^T @ lx with
    ly = [y0, y1, |y|^2, 1], lx = [-2*x0, -2*x1, 1, |x|^2] -> PSUM fp32.
  * einsum is x-major: for each 128-wide x tile, the kernel-value tile
    t_j[sensors, x_tile] is the stationary matmul operand and the tiny
    uw_j[sensors, 8] = a_j*u tensor is the moving operand, accumulating
    out[x_tile, c] over units j in PSUM.  Per-core output is [128, B*64]
    fp32 = (x%128, b, x//128, c), summed over cores on the host.
  * Sensor-sharded: each of the 8 cores handles 128 sensors, all batches.
"""

import numpy as np
import ml_dtypes

bf16 = ml_dtypes.bfloat16

B, S, X, C, D = 4, 1024, 1024, 8, 2
WIDTH, DEPTH = 16, 4
N_CORES = 8
SPC = S // N_CORES  # sensors per core = 128
NTILE = X // 128    # x tiles per batch = 8

N_WARMUP = 5        # PE warm-up matmuls (p-state ramp) before real work


def _f_exact(r, W_in, b_in, W_hid, b_hid, W_out, b_out):
    """Reference scalar function f(r), evaluated in float64 on the host."""
    h = r[..., None] * W_in[0].astype(np.float64) + b_in.astype(np.float64)
    for l in range(DEPTH):
        h = np.tanh(h @ W_hid[l].astype(np.float64)
                    + b_hid[l].astype(np.float64)) + h
    return (h @ W_out.astype(np.float64))[..., 0] + np.float64(b_out[0])


def _units_from_q(q32, thetas):
    """bf16-rounded unit tensors [q, relu(q-theta_k)...] given fp32 q."""
    units = [q32.astype(bf16).astype(np.float32)]
    qb = units[0]
    for th in thetas:
        units.append(np.maximum(qb - np.float32(th), 0.0)
                     .astype(bf16).astype(np.float32))
    return units


def _fit_basis(r_hw_flat, fr_flat, thetas0, s0_0):
    """Fit f(r) ~ c0 + c1*r + sum_j a_j * unit_j(q) on a sample of actual
    (r, f(r)) pairs, with the bf16 pipeline rounding simulated.

    Returns (s0, thetas, cvec, rms) where cvec = [c0, c1, a_0, a_1, ...].
    """
    r32 = r_hw_flat.astype(np.float32)
    r64 = r_hw_flat.astype(np.float64)

    def design(s0, thetas):
        q32 = np.exp(np.float32(-s0) * r32, dtype=np.float32)
        units = _units_from_q(q32, thetas)
        return np.stack([np.ones_like(r64), r64]
                        + [t.astype(np.float64) for t in units], 1)

    def solve(s0, thetas, lam=1e-7):
        A = design(s0, thetas)
        n = A.shape[1]
        G = A.T @ A + lam * len(r64) * np.eye(n)
        cvec = np.linalg.solve(G, A.T @ fr_flat)
        rms = float(np.sqrt(np.mean((A @ cvec - fr_flat) ** 2)))
        return cvec, rms

    from scipy.optimize import minimize

    def obj(p):
        s0 = float(np.exp(p[0]))
        return solve(s0, p[1:])[1]

    x0 = np.r_[np.log(s0_0), thetas0]
    res = minimize(obj, x0, method='Nelder-Mead',
                   options={'maxiter': 120, 'xatol': 5e-4, 'fatol': 1e-8})
    p = res.x
    s0 = float(np.exp(p[0]))
    thetas = [float(t) for t in p[1:]]
    cvec, rms = solve(s0, thetas)
    return s0, thetas, cvec, rms


def _build_and_run(lyx_np, uw_np, s0, thetas):
    """Build the Bass program and run it on the 8 cores."""
    import concourse.bass as bass
    import concourse.mybir as mybir
    from concourse.bass_utils import run_bass_kernel_spmd

    AF = mybir.ActivationFunctionType
    ALU = mybir.AluOpType
    W = 1 + len(thetas)
    NDV = len(thetas)           # DVE ops per batch

    nc = bass.Bass()
    lyx_d = nc.declare_dram_parameter(
        "lyx", [4, B * SPC + B * X], mybir.dt.bfloat16, isOutput=False)
    uw_d = nc.declare_dram_parameter(
        "uw", [SPC, B * W * C], mybir.dt.bfloat16, isOutput=False)
    o_d = nc.declare_dram_parameter(
        "o", [B, 128, NTILE * C], mybir.dt.float32, isOutput=True)

    LY0 = 0                # ly columns 0..B*SPC
    LX0 = B * SPC          # lx columns start

    from contextlib import ExitStack

    with ExitStack() as ctx:
        lyx = ctx.enter_context(
            nc.sbuf_tensor([4, B * SPC + B * X], mybir.dt.bfloat16))
        uw = ctx.enter_context(nc.sbuf_tensor([SPC, B * W * C], mybir.dt.bfloat16))
        wly = ctx.enter_context(nc.sbuf_tensor([4, SPC], mybir.dt.bfloat16))
        wlx = ctx.enter_context(nc.sbuf_tensor([4, 512], mybir.dt.bfloat16))
        qs = [ctx.enter_context(nc.sbuf_tensor(f"q{b}", [SPC, X], mybir.dt.bfloat16))
              for b in range(B)]
        tus = [[ctx.enter_context(
            nc.sbuf_tensor(f"t{k}_{b}", [SPC, X], mybir.dt.bfloat16))
            for k in range(NDV)] for b in range(B)]
        prs = [ctx.enter_context(nc.psum_tensor(f"pr{i}", [SPC, X], mybir.dt.float32))
               for i in range(3)]
        po = ctx.enter_context(
            nc.psum_tensor("po", [128, B * NTILE * C], mybir.dt.float32))
        ps = ctx.enter_context(
            nc.psum_tensor("ps", [128, 512], mybir.dt.float32))
        ob = ctx.enter_context(
            nc.sbuf_tensor([128, B * NTILE * C], mybir.dt.float32))
        s_in = ctx.enter_context(nc.semaphore("s_in"))
        s_uw = ctx.enter_context(nc.semaphore("s_uw"))
        s_r = ctx.enter_context(nc.semaphore("s_r"))
        s_q = ctx.enter_context(nc.semaphore("s_q"))
        s_dv = ctx.enter_context(nc.semaphore("s_dv"))
        s_e = ctx.enter_context(nc.semaphore("s_e"))
        s_ob = ctx.enter_context(nc.semaphore("s_ob"))
        s_out = ctx.enter_context(nc.semaphore("s_out"))
        s_wm = ctx.enter_context(nc.semaphore("s_wm"))
        block = ctx.enter_context(nc.Block())

        OBC = B * NTILE * C  # 256 output cols

        @block.sync
        def _(sync):
            # one combined ly+lx load: single DGE latency on the critical path
            sync.dma_start(out=lyx[:], in_=lyx_d[:]).then_inc(s_in, 16)
            for b in range(B):
                sync.wait_ge(s_ob, b + 1)
                sync.dma_start(out=o_d[b],
                               in_=ob[:, 64 * b:64 * (b + 1)]
                               ).then_inc(s_out, 16)

        @block.scalar
        def _(scalar):
            scalar.dma_start(out=uw[:], in_=uw_d[:]).then_inc(s_uw, 16)
            for b in range(B):
                scalar.wait_ge(s_r, b + 1)
                scalar.activation(qs[b][:], prs[b % 3][:], AF.Exp,
                                  scale=float(-s0)).then_inc(s_q, 1)
            # tail copies for the last two batches run on the (now idle)
            # scalar engine so the vector engine can stay on unit work.
            for b in (2, 3):
                scalar.wait_ge(s_e, b + 1)
                scalar.wait_ge(s_ob, b)  # order vs the vector-engine copies
                scalar.activation(ob[:, 64 * b:64 * (b + 1)],
                                  po[:, 64 * b:64 * (b + 1)],
                                  AF.Copy).then_inc(s_ob, 1)

        @block.tensor
        def _(tensor):
            # p-state warm-up: matmuls on scratch SBUF (contents irrelevant,
            # result overwritten by the batch-0 r matmul below).
            tensor.wait_ge(s_wm, 1)

            def spacer():
                # one wide matmul between bursts of 8-column matmuls: long
                # runs of tiny matmuls (>~50) hard-fault the PE front-end.
                tensor.matmul(ps[:], wly[:], wlx[:], start=True, stop=True)

            for i in range(N_WARMUP):
                spacer()

            def emit_r(b):
                pr = prs[b % 3]
                for h in range(2):
                    mm = tensor.matmul(
                        pr[:, 512 * h:512 * (h + 1)],
                        lyx[:, LY0 + SPC * b:LY0 + SPC * (b + 1)],
                        lyx[:, LX0 + X * b + 512 * h:LX0 + X * b + 512 * (h + 1)],
                        start=True, stop=True)
                    if h == 1:
                        mm.then_inc(s_r, 1)

            def emit_e(b):
                # HW requires contiguous PSUM accumulation groups: run all W
                # units of one x tile back-to-back (t outer, j inner).
                tensor.wait_ge(s_q, b + 1)
                tensor.wait_ge(s_dv, NDV * (b + 1))
                for t in range(NTILE):
                    for j in range(W):
                        src = qs[b] if j == 0 else tus[b][j - 1]
                        e = b * W + j
                        mm = tensor.matmul(
                            po[:, 64 * b + 8 * t:64 * b + 8 * t + 8],
                            src[:, 128 * t:128 * (t + 1)],
                            uw[:, C * e:C * (e + 1)],
                            start=(j == 0), stop=(j == W - 1))
                        if j == W - 1 and t == NTILE - 1:
                            mm.then_inc(s_e, 1)

            tensor.wait_ge(s_in, 16)
            emit_r(0)
            emit_r(1)
            emit_r(2)
            tensor.wait_ge(s_uw, 16)
            emit_e(0)
            tensor.wait_ge(s_q, 1)   # pr[0] drained by exp0
            emit_r(3)
            emit_e(1)
            spacer()
            emit_e(2)
            spacer()
            emit_e(3)

        @block.vector
        def _(vector):
            vector.memset(wly[:], 0.25)
            vector.memset(wlx[:], 0.25).then_inc(s_wm, 1)
            for b in range(B):
                vector.wait_ge(s_q, b + 1)
                for k, th in enumerate(thetas):
                    vector.tensor_scalar(
                        tus[b][k][:], qs[b][:], float(th), 0.0,
                        ALU.subtract, ALU.max).then_inc(s_dv, 1)
                if b in (1, 2):
                    # copy batch b-1's einsum tile out of PSUM while the
                    # scalar engine is still busy with exps; batches 2 and 3
                    # are copied by the scalar engine after its exps finish.
                    bb = b - 1
                    vector.wait_ge(s_e, bb + 1)
                    vector.tensor_copy(ob[:, 64 * bb:64 * (bb + 1)],
                                       po[:, 64 * bb:64 * (bb + 1)]
                                       ).then_inc(s_ob, 1)

    in_maps = []
    for core in range(N_CORES):
        in_maps.append({"lyx": lyx_np[core], "uw": uw_np[core]})
    res = run_bass_kernel_spmd(nc, in_maps, list(range(N_CORES)))
    global LAST_RESULT
    LAST_RESULT = res
    return res


LAST_RESULT = None


def kernel(yu, x, W_in, b_in, W_hid, b_hid, W_out, b_out):
    yu = np.asarray(yu, dtype=np.float32)
    x = np.asarray(x, dtype=np.float32)
    f_args = (np.asarray(W_in, np.float32), np.asarray(b_in, np.float32),
              np.asarray(W_hid, np.float32), np.asarray(b_hid, np.float32),
              np.asarray(W_out, np.float32), np.asarray(b_out, np.float32))

    y = yu[:, :, -D:].astype(np.float64)   # (B,S,2) sensor positions
    u = yu[:, :, :C].astype(np.float64)    # (B,S,C) sensor values
    xd = x.astype(np.float64)              # (B,X,2)

    # ---- the r the hardware will see (bf16 factors, fp32 accumulate) ----
    yb = y.astype(bf16).astype(np.float32)
    xb2 = (xd ** 2).sum(-1).astype(bf16).astype(np.float32)      # (B,X)
    yb2 = (y ** 2).sum(-1).astype(bf16).astype(np.float32)       # (B,S)
    m2x = (-2.0 * xd).astype(bf16).astype(np.float32)            # (B,X,2)
    r_hw = (np.einsum('bsd,bxd->bsx', yb, m2x, dtype=np.float32)
            + yb2[:, :, None] + xb2[:, None, :]).astype(np.float32)

    # ---- runtime fit of the relu-in-q basis against exact f ----
    rng = np.random.default_rng(12345)
    idx = rng.choice(r_hw.size, 200_000, replace=False)
    r_s = r_hw.reshape(-1)[idx].astype(np.float64)
    fr_s = _f_exact(r_s, *f_args)

    s0, thetas, cvec, rms = _fit_basis(
        r_s, fr_s, thetas0=np.array([0.92, 0.67, 0.29]), s0_0=0.33)
    # guarded fallback: a denser knot set if the 3-knot fit is poor
    if rms > 4.5e-2:
        s0b, thb, cvb, rmsb = _fit_basis(
            r_s, fr_s, thetas0=np.array([0.95, 0.82, 0.6, 0.38, 0.18]),
            s0_0=0.28)
        if rmsb < rms:
            s0, thetas, cvec, rms = s0b, thb, cvb, rmsb
    c0, c1, amps = float(cvec[0]), float(cvec[1]), cvec[2:]
    W = 1 + len(thetas)

    # ---- host-side packing ----
    # lyx: [4, B*SPC + B*X] bf16 per core: ly block then lx block
    lyx_np = np.zeros((N_CORES, 4, B * SPC + B * X), bf16)
    uw_np = np.empty((N_CORES, SPC, B * W * C), bf16)
    LX0 = B * SPC
    for core in range(N_CORES):
        sl = slice(core * SPC, (core + 1) * SPC)
        for b in range(B):
            cy = slice(SPC * b, SPC * (b + 1))
            lyx_np[core, 0, cy] = yb[b, sl, 0].astype(bf16)
            lyx_np[core, 1, cy] = yb[b, sl, 1].astype(bf16)
            lyx_np[core, 2, cy] = yb2[b, sl].astype(bf16)
            lyx_np[core, 3, cy] = bf16(1.0)
            ub = u[b, sl]                                    # (128,C)
            for j in range(W):
                e = b * W + j
                uw_np[core, :, C * e:C * (e + 1)] = \
                    (amps[j] * ub).astype(bf16)
    for b in range(B):
        cx = slice(LX0 + X * b, LX0 + X * (b + 1))
        lyx_np[:, 0, cx] = m2x[b, :, 0].astype(bf16)
        lyx_np[:, 1, cx] = m2x[b, :, 1].astype(bf16)
        lyx_np[:, 2, cx] = bf16(1.0)
        lyx_np[:, 3, cx] = xb2[b].astype(bf16)

    res = _build_and_run(lyx_np, uw_np, s0, thetas)

    # ---- host-side unshard: sum sensor shards, unpack x-major layout ----
    acc = np.zeros((B, 128, NTILE * C), np.float64)
    for core in range(N_CORES):
        acc += res.results[core]["o"].astype(np.float64)
    # acc[b, xl, 8*t + c] -> out[b, 128*t + xl, c]
    out = acc.reshape(B, 128, NTILE, C).transpose(0, 2, 1, 3).reshape(B, X, C)
    out = out / S

    # exact affine contribution (1/S)*sum_s u[b,s,c]*(c0 + c1*r[b,s,x]):
    # r = |x|^2 - 2 x.y + |y|^2 gives a rank-2 structure over sensors.
    su = u.sum(1)                                   # (B,C)
    sur2 = np.einsum('bsc,bs->bc', u, (y ** 2).sum(-1))
    suy = np.einsum('bsc,bsd->bcd', u, y)           # (B,C,2)
    x2 = (xd ** 2).sum(-1)                          # (B,X)
    aff = (c0 * su[:, None, :]
           + c1 * (x2[:, :, None] * su[:, None, :]
                   + sur2[:, None, :]
                   - 2.0 * np.einsum('bxd,bcd->bxc', xd, suy))) / S
    return (out + aff).astype(np.float32)
